# revision 8
# baseline (speedup 1.0000x reference)
"""Trainium2 Bass kernel for nn_LsqNonneg: batched NNLS.

Algorithm: constant-momentum accelerated projected gradient (converges to the
same NNLS KKT point the reference's 200-iteration FISTA approaches):

    AtA = A.T @ A;  L, mu = extreme eigenvalues;  step = 1/L
    W  = I - step*AtA;  beta = (sqrt(L/mu)-1)/(sqrt(L/mu)+1)
    B  = step * A.T @ X
    S_1 = relu(B); S_0 = 0
    for k = 1..K-1:
        S_{k+1} = relu( [(1+beta)W] S_k + [-beta W] S_{k-1} + B )
    return S_K

Both weight matrices are FIXED -> kept in SBUF, no per-iteration weight DMA.
fp32r matmuls round each operand to 11-bit mantissa; the deterministic bias
from rounding the fixed weights is suppressed by dithering: n=8 pre-rounded
variants per weight whose per-entry mean equals the exact value, cycled in a
balanced shuffled schedule.

Device layout (per core, ns=4096 columns): S packed [128, q=1024]; partition
group g holds columns [g*q,(g+1)*q). Weights are block-diagonal diag4 so one
full-array matmul advances all 4 groups. Per iteration, per 512-col slice:
3 accumulating matmuls into one PSUM bank (ident@B, Wa@S_k, Wb@S_{k-1});
relu(psum)->S on ScalarE (slice 0) / VectorE (slice 1).
"""

import os
import sys

import numpy as np

for _p in ("/opt/trn_rl_repo", "/root/.axon_site/_ro/trn_rl_repo"):
    if os.path.isdir(_p) and _p not in sys.path:
        sys.path.append(_p)

from contextlib import ExitStack

import concourse.bass as bass
import concourse.bacc as bacc
import concourse.tile as tile
from concourse import mybir
from concourse.bass_utils import run_bass_kernel_spmd

M, KD, N_FULL, N_CORES = 512, 32, 32768, 8
ITERS = 72           # total iterations (S_ITERS is returned)
N_DITHER = 8
DITHER_SEED = 1

F32 = mybir.dt.float32
F32R = mybir.dt.float32r

LAST_RESULTS = None  # BassKernelResults of the most recent run (for test.py)


def build_program(ns: int, iters: int, n_dither: int):
    q = ns // 4          # free extent of the packed [128, q] S layout
    nsl = q // 512       # 512-wide slices (one PSUM bank each)
    assert ns % 2048 == 0 and nsl >= 1

    nc = bacc.Bacc("TRN2", target_bir_lowering=False)

    x_d = nc.dram_tensor("x", [M, ns], F32, kind="ExternalInput")
    apad_d = nc.dram_tensor("apad", [4, M, 128], F32, kind="ExternalInput")
    wd_d = nc.dram_tensor("wd", [n_dither, 2, 128, 128], F32,
                          kind="ExternalInput")
    id_d = nc.dram_tensor("ident", [128, 128], F32, kind="ExternalInput")
    out_d = nc.dram_tensor("s_out", [KD, ns], F32, kind="ExternalOutput")

    sched = _dither_schedule(iters, n_dither)

    with ExitStack() as ctx:
        tc = ctx.enter_context(tile.TileContext(nc))
        persist = ctx.enter_context(tc.tile_pool(name="persist", bufs=1))
        xpool = ctx.enter_context(tc.tile_pool(name="xstage", bufs=4))
        psum = ctx.enter_context(tc.tile_pool(name="psum", bufs=4,
                                              space="PSUM"))

        # weights go on the scalar HWDGE ring so X can stream on the sync
        # ring in parallel; each is one consolidated DMA.
        id_sb = persist.tile([128, 128], F32R)
        nc.scalar.dma_start(id_sb[:], id_d[:].bitcast(F32R))

        # dither variants: (i, j) block at free offset 128*(2i+j)
        w_sb = persist.tile([128, 2 * n_dither * 128], F32R)
        nc.scalar.dma_start(
            w_sb[:].rearrange("p (i j m) -> p i j m", i=n_dither, j=2),
            wd_d[:].rearrange("i j p m -> p i j m").bitcast(F32R))

        apc = persist.tile([128, 16 * 128], F32R)  # (g,c) chunk at 128*(4g+c)
        nc.scalar.dma_start(
            apc[:].rearrange("p (g c m) -> p g c m", g=4, c=4),
            apad_d[:].rearrange("g (c p) m -> p g c m", p=128).bitcast(F32R))

        b_sb = persist.tile([128, q], F32R)
        s_st = [persist.tile([128, q], F32R, name=f"s_st{i}")
                for i in range(3)]

        # ---- prologue: B = (step A).T @ X in packed layout; S_1 = relu(B) ----
        pb = psum.tile([128, q], F32, tag="pt")
        for c in range(4):
            xt = xpool.tile([128, ns], F32R)
            eng = nc.sync if c % 2 == 0 else nc.scalar
            eng.dma_start(xt[:], x_d[128 * c:128 * (c + 1), :].bitcast(F32R))
            for g in range(4):
                lhs = apc[:, 128 * (4 * g + c):128 * (4 * g + c + 1)]
                for s in range(nsl):
                    nc.tensor.matmul(
                        pb[:, 512 * s:512 * (s + 1)],
                        lhs,
                        xt[:, g * q + 512 * s: g * q + 512 * (s + 1)],
                        start=(c == 0 and g == 0),
                        stop=(c == 3 and g == 3),
                    )
        for s in range(nsl):
            sl = slice(512 * s, 512 * (s + 1))
            if s % 2 == 0:
                nc.scalar.copy(b_sb[:, sl], pb[:, sl])
                nc.vector.tensor_scalar_max(s_st[1][:, sl], pb[:, sl], 0.0)
            else:
                nc.vector.tensor_copy(b_sb[:, sl], pb[:, sl])
                nc.scalar.activation(s_st[1][:, sl], pb[:, sl],
                                     mybir.ActivationFunctionType.Relu)

        # ---- loop: k = 1..iters-1 computes S_{k+1} ----
        for k in range(1, iters):
            i = sched[k]
            wa = w_sb[:, 128 * (2 * i):128 * (2 * i + 1)]
            wb = w_sb[:, 128 * (2 * i + 1):128 * (2 * i + 2)]
            cur = s_st[k % 3]
            prev = s_st[(k - 1) % 3]
            dest = s_st[(k + 1) % 3]
            pt = psum.tile([128, q], F32, tag="pt")
            sls = [slice(512 * s, 512 * (s + 1)) for s in range(nsl)]
            # same-weight matmuls grouped so the PE reloads weights only 3x
            # per iteration (ident, wa, wb); ident@B goes first since it has
            # no relu dependency.
            for sl in sls:
                nc.tensor.matmul(pt[:, sl], id_sb[:], b_sb[:, sl],
                                 start=True, stop=False)
            for sl in sls:
                nc.tensor.matmul(pt[:, sl], wa, cur[:, sl],
                                 start=False, stop=(k == 1))
            if k > 1:
                for sl in sls:
                    nc.tensor.matmul(pt[:, sl], wb, prev[:, sl],
                                     start=False, stop=True)
            for s, sl in enumerate(sls):
                if s % 2 == 0:
                    nc.scalar.activation(dest[:, sl], pt[:, sl],
                                         mybir.ActivationFunctionType.Relu)
                else:
                    nc.vector.tensor_scalar_max(dest[:, sl], pt[:, sl], 0.0)

        final = s_st[iters % 3]
        for g in range(4):
            for s in range(nsl):
                nc.sync.dma_start(
                    out_d[:, g * q + 512 * s: g * q + 512 * (s + 1)],
                    final[32 * g:32 * (g + 1),
                          512 * s:512 * (s + 1)].bitcast(F32))

    nc.finalize()
    return nc


def _dither_schedule(iters, n):
    sched = np.concatenate([np.arange(n)] * (iters // n + 2))[:iters]
    rng = np.random.default_rng(DITHER_SEED)
    rng.shuffle(sched)
    return sched


def _round11(x):
    u = np.ascontiguousarray(np.asarray(x, dtype=np.float32)).view(np.uint32)
    u = ((u + np.uint32(1 << 11)) >> np.uint32(12)) << np.uint32(12)
    return u.view(np.float32).astype(np.float64)


def _dither_variants(Mx, n):
    """n 11-bit-exact matrices whose per-entry mean ~= Mx."""
    M64 = np.asarray(Mx, dtype=np.float64)
    hi = _round11(M64)
    ulp = 2.0 ** (np.floor(np.log2(np.abs(M64) + 1e-300)) - 11)
    flo = np.where(hi > M64, hi - ulp, hi)
    fhi = flo + ulp
    frac = np.clip((M64 - flo) / ulp, 0, 1)
    cnt = np.rint(frac * n).astype(int)
    return [np.where(i < cnt, fhi, flo).astype(np.float32) for i in range(n)]


def host_prep(A: np.ndarray, n_dither: int):
    A64 = np.asarray(A, dtype=np.float64)
    AtA = A64.T @ A64
    ev = np.linalg.eigvalsh(AtA)
    L, mu = ev[-1], ev[0]
    step = 1.0 / L
    W = np.eye(KD) - step * AtA
    beta = (np.sqrt(L / mu) - 1.0) / (np.sqrt(L / mu) + 1.0)

    was = _dither_variants(((1.0 + beta) * W).T, n_dither)
    wbs = _dither_variants((-beta * W).T, n_dither)
    wd = np.zeros((n_dither, 2, 128, 128), dtype=np.float32)
    for i in range(n_dither):
        for g in range(4):
            blk = slice(32 * g, 32 * (g + 1))
            wd[i, 0][blk, blk] = was[i]
            wd[i, 1][blk, blk] = wbs[i]

    As = (step * A64).astype(np.float32)
    apad = np.zeros((4, M, 128), dtype=np.float32)
    for g in range(4):
        apad[g, :, 32 * g:32 * (g + 1)] = As
    ident = np.eye(128, dtype=np.float32)
    return wd, apad, ident


_PROGRAM_CACHE = {}


def _get_program(ns, iters, n_dither):
    key = (ns, iters, n_dither)
    if key not in _PROGRAM_CACHE:
        _PROGRAM_CACHE[key] = build_program(ns, iters, n_dither)
    return _PROGRAM_CACHE[key]


def kernel(X: np.ndarray, A: np.ndarray) -> np.ndarray:
    global LAST_RESULTS
    X = np.ascontiguousarray(np.asarray(X, dtype=np.float32))
    A = np.ascontiguousarray(np.asarray(A, dtype=np.float32))
    assert X.shape == (M, N_FULL) and A.shape == (M, KD)

    ns = N_FULL // N_CORES
    wd, apad, ident = host_prep(A, N_DITHER)
    nc = _get_program(ns, ITERS, N_DITHER)

    in_maps = []
    for c in range(N_CORES):
        in_maps.append({
            "x": np.ascontiguousarray(X[:, c * ns:(c + 1) * ns]),
            "apad": apad,
            "wd": wd,
            "ident": ident,
        })

    res = run_bass_kernel_spmd(nc, in_maps, core_ids=list(range(N_CORES)))
    LAST_RESULTS = res
    S = np.concatenate([res.results[c]["s_out"] for c in range(N_CORES)], axis=1)
    return np.ascontiguousarray(S.astype(np.float32))
